# revision 1
# baseline (speedup 1.0000x reference)
"""Bass/Trainium2 kernel for nn_BalancedCELoss (8 NeuronCores, SPMD).

Sharding: 8 cores = B(2) x Z-quarters(4). Each core processes a probs slab
[16, 24, 96, 96] and computes on-device:
  - entropy partial  sum(p * ln p)          (ScalarE Ln + VectorE fused mul-reduce)
  - sum0_z / sum0_y / sum0_dense planes     (TensorE row-pass matmuls with
    per-row-octet block-diagonal weight tables, (c,g)-packed 128 partitions)
Host finishes with the E-sized tail: sum0_x einsum, target gather, focal,
masked per-slice reductions, final means.
"""
import sys, os
sys.path.insert(0, "/opt/trn_rl_repo")

import numpy as np
from contextlib import ExitStack

import concourse.bass as bass
import concourse.mybir as mybir
from concourse.tile import TileContext
from concourse.bass_utils import run_bass_kernel_spmd

EPS = 1e-6
GAMMA = 2.0
MULT = 3.0

B, C, Z, Y, X = 2, 16, 96, 96, 96
ZQ = 4                 # z-quarters per sample
ZC = Z // ZQ           # 24 z-slices per core
ROWS = ZC * Y          # 2304 (z,y) rows per core
NOCT = ROWS // 8       # 288 row-octets
NSUP = NOCT // 16      # 18 supertiles (16 octets each)
ENTC = NSUP            # entropy accum columns
OUTW = ENTC + NSUP * 384

_CACHE = {}


def _build_nc():
    nc = bass.Bass()
    # host pre-arranges: p[s, c*8+g, j*96+x] = probs[c, row=s*128+g*16+j, x]
    p_in = nc.declare_dram_parameter("p", [NSUP, 128, 16 * 96], mybir.dt.float32, isOutput=False)
    lt_in = nc.declare_dram_parameter("ltab", [128, NSUP * 16 * 24], mybir.dt.float32, isOutput=False)
    out = nc.declare_dram_parameter("out", [128, OUTW], mybir.dt.float32, isOutput=True)

    F = 16 * 96
    with ExitStack() as ctx:
        lt_all = ctx.enter_context(nc.sbuf_tensor([128, NSUP * 16 * 24], mybir.dt.float32))
        pt0 = ctx.enter_context(nc.sbuf_tensor([128, F], mybir.dt.float32))
        pt1 = ctx.enter_context(nc.sbuf_tensor([128, F], mybir.dt.float32))
        lg0 = ctx.enter_context(nc.sbuf_tensor([128, F], mybir.dt.float32))
        lg1 = ctx.enter_context(nc.sbuf_tensor([128, F], mybir.dt.float32))
        sc = ctx.enter_context(nc.sbuf_tensor([128, F], mybir.dt.float32))
        outsb = ctx.enter_context(nc.sbuf_tensor([128, OUTW], mybir.dt.float32))
        ps0 = ctx.enter_context(nc.psum_tensor([128, 384], mybir.dt.float32))
        ps1 = ctx.enter_context(nc.psum_tensor([128, 384], mybir.dt.float32))
        sd = ctx.enter_context(nc.semaphore("sd"))
        sm = ctx.enter_context(nc.semaphore("sm"))
        sl = ctx.enter_context(nc.semaphore("sl"))
        ss = ctx.enter_context(nc.semaphore("ss"))
        se = ctx.enter_context(nc.semaphore("se"))
        block = ctx.enter_context(nc.Block())
        pts = [pt0, pt1]
        lgs = [lg0, lg1]
        pss = [ps0, ps1]

        @block.sync
        def _(sync):
            sync.dma_start(out=lt_all[:, :], in_=lt_in[:, :]).then_inc(sd, 16)
            for s in range(NSUP):
                if s >= 2:
                    sync.wait_ge(sm, s - 1)
                    sync.wait_ge(sl, s - 1)
                    sync.wait_ge(ss, s - 1)
                sync.dma_start(out=pts[s % 2][:, :], in_=p_in[s]).then_inc(sd, 16)
            sync.wait_ge(ss, NSUP)
            sync.wait_ge(se, NSUP)
            sync.dma_start(out=out[:, :], in_=outsb[:, :]).then_inc(sd, 16)

        @block.tensor
        def _(tensor):
            tensor.wait_ge(sd, 16)
            for s in range(NSUP):
                tensor.wait_ge(sd, 16 * (s + 2))
                if s >= 2:
                    tensor.wait_ge(se, s - 1)
                pt, ps = pts[s % 2], pss[s % 2]
                for j in range(16):
                    q, cb = j % 4, j // 4
                    w0 = (s * 16 + j) * 24
                    mm = tensor.matmul(
                        ps[q * 32:q * 32 + 24, cb * 96:(cb + 1) * 96],
                        lt_all[:, w0:w0 + 24], pt[:, j * 96:(j + 1) * 96],
                        start=True, stop=True, tile_position=(0, q * 32))
                mm.then_inc(sm, 1)

        @block.scalar
        def _(scalar):
            for s in range(NSUP):
                scalar.wait_ge(sd, 16 * (s + 2))
                if s >= 2:
                    scalar.wait_ge(ss, s - 1)
                scalar.activation(lgs[s % 2][:, :], pts[s % 2][:, :],
                                  mybir.ActivationFunctionType.Ln).then_inc(sl, 1)

        @block.vector
        def _(vector):
            for s in range(NSUP):
                vector.wait_ge(sl, s + 1)
                vector.wait_ge(sd, 16 * (s + 2))
                vector.scalar_tensor_tensor(
                    sc[:, :], lgs[s % 2][:, :], 0.0, pts[s % 2][:, :],
                    mybir.AluOpType.bypass, mybir.AluOpType.mult,
                    accum_out=outsb[:, s:s + 1]).then_inc(ss, 1)
                vector.wait_ge(sm, s + 1)
                vector.tensor_copy(
                    outsb[:, ENTC + s * 384:ENTC + (s + 1) * 384],
                    pss[s % 2][:, :]).then_inc(se, 1)
    return nc


def _focal(x):
    return -(1.0 - x) ** GAMMA * np.log(np.clip(x, EPS, 1.0 - EPS))


def kernel(probs, target, annotated_fg_categories, annotated_categories_z_axis,
           annotated_categories_y_axis, annotated_categories_x_axis, masks, is_sparse):
    probs = np.asarray(probs, np.float32)
    target = np.asarray(target, np.int32)
    masks = np.asarray(masks, np.int32)
    is_sparse = np.asarray(is_sparse, np.int32)
    afc = np.asarray(annotated_fg_categories, np.int32)
    az = np.asarray(annotated_categories_z_axis, np.int32)
    ay = np.asarray(annotated_categories_y_axis, np.int32)
    ax = np.asarray(annotated_categories_x_axis, np.int32)

    # per-sample unannotated indicators (float weights for the contraction)
    un_z = (az <= 0).astype(np.float32)          # [B, Z, C]
    un_y = (ay <= 0).astype(np.float32)          # [B, Y, C]
    ks = np.arange(C)
    annot = np.any((afc[:, :, None] == ks[None, None, :]) & (afc[:, :, None] > 0), axis=1)
    un_d = (~annot).astype(np.float32)           # [B, C]
    un_x = (ax <= 0).astype(np.float32)          # [B, X, C]

    if "nc" not in _CACHE:
        _CACHE["nc"] = _build_nc()
    nc = _CACHE["nc"]

    in_maps = []
    for core in range(8):
        b, zq = core // ZQ, core % ZQ
        slab = probs[b, :, zq * ZC:(zq + 1) * ZC]          # [C, ZC, Y, X]
        # [s, c*8+g, j*96+x]
        slab = np.ascontiguousarray(
            slab.reshape(C, NSUP, 8, 16, X).transpose(1, 0, 2, 3, 4)
        ).reshape(NSUP, 128, 16 * 96)
        # ltab[s, c*8+g, j, g*3+a] = w_a[c, row=(s*16+j)*8+g]
        r = np.arange(ROWS)
        zs = zq * ZC + r // 96
        ysl = r % 96
        wz = un_z[b][zs, :].T                    # [C, ROWS]
        wy = un_y[b][ysl, :].T
        wd = np.broadcast_to(un_d[b][:, None], (C, ROWS))
        Wa = np.stack([wz, wy, wd], 0)           # [3, C, ROWS]
        ltab = np.zeros((NSUP, 128, 16, 24), np.float32)
        s_i, g_i, j_i = r // 128, (r % 128) // 16, r % 16
        for a in range(3):
            for c in range(C):
                ltab[s_i, c * 8 + g_i, j_i, g_i * 3 + a] = Wa[a, c]
        ltab = np.ascontiguousarray(ltab.transpose(1, 0, 2, 3)).reshape(128, NSUP * 16 * 24)
        in_maps.append({"p": slab, "ltab": ltab})

    _CACHE["in_maps"] = in_maps
    res = run_bass_kernel_spmd(nc, in_maps, core_ids=list(range(8)))
    outs = [r["out"] for r in res.results]

    # ---- host finish -------------------------------------------------------
    fg_all = target > 0
    p_t = np.take_along_axis(probs, target[:, None].astype(np.int64), axis=1)[:, 0]
    ce_fg_all = _focal(p_t)

    ce_list, has_list, reg_list = [], [], []
    for b in range(B):
        ent_sum = 0.0
        sum0 = {k: np.empty((ZC * ZQ, Y, X), np.float32) for k in "zyd"}
        for zq in range(ZQ):
            o = np.asarray(outs[b * ZQ + zq], np.float32)
            ent_sum += float(o[:, :ENTC].sum())
            blk = o[:, ENTC:].reshape(128, NSUP, 384).transpose(1, 0, 2)
            m = blk.reshape(NSUP, 4, 32, 4, 96)[:, :, :24]   # [s, quad, (g,a), colblk, x]
            m = m.reshape(NSUP, 4, 8, 3, 4, 96)              # [s, quad, g, a, colblk, x]
            # j = colblk*4 + quad ; row = s*128 + g*16 + j
            m = m.transpose(3, 0, 2, 4, 1, 5)            # [a, s, g, colblk, quad, x]
            rows = m.reshape(3, ROWS, X)
            for ai, k in enumerate("zyd"):
                sum0[k][zq * ZC:(zq + 1) * ZC] = rows[ai].reshape(ZC, Y, X)

        Vfull = Z * Y * X
        ent = -ent_sum / Vfull
        t = target[b]
        reg = MULT * ent if np.all(t == 0) else ent
        fg = fg_all[b]
        ce_fg = ce_fg_all[b]
        sum0_x = np.einsum("czyx,xc->zyx", probs[b], un_x[b], optimize=True)

        mask = masks[b]
        valid = {
            "z": mask.sum(axis=(1, 2)) == Y * X,
            "y": mask.sum(axis=(0, 2)) == Z * X,
            "x": mask.sum(axis=(0, 1)) == Z * Y,
        }
        hasfg = {
            "z": fg.any(axis=(1, 2)), "y": fg.any(axis=(0, 2)), "x": fg.any(axis=(0, 1)),
        }
        shp = {"z": (Z, 1, 1), "y": (1, Y, 1), "x": (1, 1, X)}
        per = {"z": float(Y * X), "y": float(Z * X), "x": float(Z * Y)}
        means, contribs = [], []
        for k, s0 in (("z", sum0["z"]), ("y", sum0["y"]), ("x", sum0_x)):
            ce = np.where(fg, ce_fg, _focal(s0))
            act = (valid[k] & hasfg[k]).astype(np.float32)
            cnt = act.sum() * per[k]
            sm = float((ce * act.reshape(shp[k])).sum())
            means.append(sm / max(cnt, 1.0))
            contribs.append(1.0 if cnt > 0 else 0.0)
        n_ax = sum(contribs)
        sparse_ce = sum(m_ * c_ for m_, c_ in zip(means, contribs)) / max(n_ax, 1.0)
        sparse_has = n_ax > 0

        dense_ce = float(np.where(fg, ce_fg, _focal(sum0["d"])).mean())

        if is_sparse[b, 0] == 1:
            ce_i, has_i = sparse_ce, 1.0 if sparse_has else 0.0
        else:
            ce_i, has_i = dense_ce, 1.0
        ce_list.append(ce_i); has_list.append(has_i); reg_list.append(reg)

    n = sum(has_list)
    ce_out = (sum(c * h for c, h in zip(ce_list, has_list)) / max(n, 1.0)) if n > 0 else 0.0
    return np.float32(ce_out), np.float32(np.mean(reg_list))

